# revision 2
# baseline (speedup 1.0000x reference)
"""Ternary-expert MLP (gate/up/silu/down) on 8 trn2 NeuronCores.

Strategy: data-parallel over tokens. Each core processes T/8 = 512 tokens
with the full (scale-folded, fp16) weight set; no collectives.

Per core, everything is computed in "transposed" space so the contraction
dim always sits on SBUF partitions:
  phase 1: for each of 44 inter tiles  g^T/u^T [128i, 512t] = W-tile.T @ x^T
           hdn^T = silu(g^T) * u^T   (fp16, stays in SBUF)
  phase 2: for each of 16 hidden tiles out^T [128h, 512t] = D-tile.T @ hdn^T

Host side folds the per-channel scales into the ternary weights (exact in
fp16 up to one rounding), packs weights so every DMA descriptor is
contiguous per partition, and un-transposes the per-core outputs.
"""

import numpy as np

HIDDEN = 2048
INTER = 5632
B, S = 2, 2048
T = B * S
NCORES = 8
TPC = T // NCORES          # 512 tokens per core
P = 128
KH = HIDDEN // P           # 16 hidden-dim k-tiles
NI = INTER // P            # 44 intermediate tiles

_cache = {}


def _build_nc(kh=KH, ni=NI, tpc=TPC):
    import concourse.bacc as bacc
    import concourse.tile as tile
    from concourse import mybir

    f16 = mybir.dt.float16
    f32 = mybir.dt.float32

    nc = bacc.Bacc("TRN2", target_bir_lowering=False, debug=False)
    xt = nc.dram_tensor("xt", [P, kh, tpc], f16, kind="ExternalInput").ap()
    gw = nc.dram_tensor("gw", [ni, P, kh * P], f16, kind="ExternalInput").ap()
    uw = nc.dram_tensor("uw", [ni, P, kh * P], f16, kind="ExternalInput").ap()
    dw = nc.dram_tensor("dw", [kh, P, ni * P], f16, kind="ExternalInput").ap()
    ot = nc.dram_tensor("ot", [kh, P, tpc], f32, kind="ExternalOutput").ap()

    with tile.TileContext(nc) as tc:
        with (
            tc.tile_pool(name="xp", bufs=1) as xp,
            tc.tile_pool(name="hp", bufs=1) as hp,
            tc.tile_pool(name="wg", bufs=3) as wg_pool,
            tc.tile_pool(name="wu", bufs=3) as wu_pool,
            tc.tile_pool(name="wd", bufs=3) as wd_pool,
            tc.tile_pool(name="act", bufs=3) as act_pool,
            tc.tile_pool(name="ob", bufs=3) as ob_pool,
            tc.tile_pool(name="ps", bufs=2, space="PSUM") as ps_pool,
            tc.tile_pool(name="po", bufs=2, space="PSUM") as po_pool,
        ):
            xsb = xp.tile([P, kh, tpc], f16)
            nc.sync.dma_start(out=xsb[:], in_=xt[:])
            hdn = hp.tile([P, ni, tpc], f16)

            for it in range(ni):
                wgt = wg_pool.tile([P, kh * P], f16)
                nc.sync.dma_start(out=wgt[:], in_=gw[it])
                wut = wu_pool.tile([P, kh * P], f16)
                nc.sync.dma_start(out=wut[:], in_=uw[it])
                pg = ps_pool.tile([P, tpc], f32)
                pu = ps_pool.tile([P, tpc], f32)
                for k in range(kh):
                    nc.tensor.matmul(
                        pg[:], wgt[:, k * P:(k + 1) * P], xsb[:, k],
                        start=(k == 0), stop=(k == kh - 1),
                    )
                for k in range(kh):
                    nc.tensor.matmul(
                        pu[:], wut[:, k * P:(k + 1) * P], xsb[:, k],
                        start=(k == 0), stop=(k == kh - 1),
                    )
                sg = act_pool.tile([P, tpc], f16)
                nc.scalar.activation(sg[:], pg[:], mybir.ActivationFunctionType.Sigmoid)
                sm = act_pool.tile([P, tpc], f16)
                nc.vector.tensor_mul(sm[:], sg[:], pg[:])
                nc.vector.tensor_mul(hdn[:, it], sm[:], pu[:])

            for hg in range(kh):
                wdt = wd_pool.tile([P, ni * P], f16)
                nc.sync.dma_start(out=wdt[:], in_=dw[hg])
                po = po_pool.tile([P, tpc], f32)
                for ik in range(ni):
                    nc.tensor.matmul(
                        po[:], wdt[:, ik * P:(ik + 1) * P], hdn[:, ik],
                        start=(ik == 0), stop=(ik == ni - 1),
                    )
                ob = ob_pool.tile([P, tpc], f32)
                nc.scalar.copy(ob[:], po[:])
                nc.sync.dma_start(out=ot[hg], in_=ob[:])

    nc.compile()
    return nc


def _pack_weights(gate_w, up_w, down_w, gate_s, up_s, down_s):
    gw = (gate_w * gate_s[:, None]).reshape(NI, P, KH, P)
    gw = np.ascontiguousarray(gw.transpose(0, 3, 2, 1)).astype(np.float16)
    uw = (up_w * up_s[:, None]).reshape(NI, P, KH, P)
    uw = np.ascontiguousarray(uw.transpose(0, 3, 2, 1)).astype(np.float16)
    dwp = (down_w * down_s[:, None]).reshape(KH, P, NI, P)
    dwp = np.ascontiguousarray(dwp.transpose(0, 3, 2, 1)).astype(np.float16)
    return (gw.reshape(NI, P, KH * P), uw.reshape(NI, P, KH * P),
            dwp.reshape(KH, P, NI * P))


def _pack_x(xf):
    # per-core x^T tiles: xt[p, k, t] = x_core[t, k*128+p]
    outs = []
    for c in range(NCORES):
        xc = xf[c * TPC:(c + 1) * TPC].reshape(TPC, KH, P)
        outs.append(np.ascontiguousarray(xc.transpose(2, 1, 0)).astype(np.float16))
    return outs


def _run(in_maps, trace=False, tmpdir=None):
    from concourse.bass_utils import run_bass_kernel_spmd

    if "nc" not in _cache:
        _cache["nc"] = _build_nc()
    return run_bass_kernel_spmd(
        _cache["nc"], in_maps, list(range(NCORES)), trace=trace, tmpdir=tmpdir,
    )


def make_in_maps(x, gate_w, up_w, down_w, gate_s, up_s, down_s):
    x = np.asarray(x, np.float32)
    gate_w = np.asarray(gate_w, np.float32)
    up_w = np.asarray(up_w, np.float32)
    down_w = np.asarray(down_w, np.float32)
    gate_s = np.asarray(gate_s, np.float32)
    up_s = np.asarray(up_s, np.float32)
    down_s = np.asarray(down_s, np.float32)

    gw, uw, dwp = _pack_weights(gate_w, up_w, down_w, gate_s, up_s, down_s)
    xts = _pack_x(x.reshape(T, HIDDEN))
    return [{"xt": xts[c], "gw": gw, "uw": uw, "dw": dwp} for c in range(NCORES)]


def unpack_out(results):
    # ot: (16, 128, 512) f32 per core; out_core[t, hg*128+p] = ot[hg, p, t]
    parts = []
    for c in range(NCORES):
        ot = results[c]["ot"]
        parts.append(ot.transpose(2, 0, 1).reshape(TPC, HIDDEN))
    return np.concatenate(parts, axis=0).reshape(B, S, HIDDEN)


def kernel(x, gate_w, up_w, down_w, gate_s, up_s, down_s):
    in_maps = make_in_maps(x, gate_w, up_w, down_w, gate_s, up_s, down_s)
    res = _run(in_maps)
    return unpack_out(res.results)
